# revision 16
# baseline (speedup 1.0000x reference)
"""DotsVisionAttention Trainium2 kernel (v1 — pipelined rewrite).

Full-input contract: kernel(**inputs) takes the unsharded tensors from
setup_inputs() and returns the full [8192, 1280] fp32 output.

Sharding: data-parallel over the 8 packed image segments (attention is
block-diagonal with 8 equal segments of 1024 tokens) — core i processes
tokens [1024*i, 1024*(i+1)), no collectives.

Differences vs v0 baseline:
  * All weight/activation DMAs are host-packed so each transfer is one
    big partition-major descriptor block (128 descriptors) — Pool/SP
    descriptor-generation time drops ~10x.
  * v is produced in natural [token, ch] layout (no PE transposes) and
    scattered into ones-augmented per-head PV operand tiles.
  * Scores matmul emits a single 1024-wide instruction into a 2-bank
    PSUM tile; exp processes 1024 columns per ScalarE instruction.
  * Softmax reciprocal on DVE (vector.reciprocal); denominator taken
    from PSUM row 96 of the PV accumulator (ones column trick).
  * proj contracts over the unpadded 1280 ctx channels (10 K-chunks).
  * Attention for head h is emitted as micro-steps interleaved between
    qkv chunk matmul groups ("pump") so the PE never stalls on the
    ScalarE exp chain and stays at max p-state.

v2 changes (this session):
  * rope's rotate_half is an SBUF->SBUF DMA swap (sign folded into the
    host-side sin table) instead of a PE permutation matmul — removes
    32k matmul columns (~14us of PE) and a psum slot per head-part.
  * prologue hT/vw0 loads split across 4 issue queues (partition
    halves) so descriptor generation runs in parallel.
  * psum->sbuf copies rebalanced: v-scatter + qk staging on DVE
    (idle during those phases), keeping ScalarE clear for the exp
    chain that gates the attention pipeline.
  * output DMAs spread across issue queues.
"""

from contextlib import ExitStack

import ml_dtypes
import numpy as np

import concourse.bass as bass
import concourse.bass_utils as _bass_utils
import concourse.tile as tile
from concourse import bacc, mybir


# NOTE: walrus's --enable-ldw-opt would dedupe the ~100ns Ldweights that
# precedes every matmul (about 170us of TensorE time here), but this build
# rejects bass-emitted Ldweights ("not compatible with LDW optimization"),
# so the only lever is emitting fewer matmuls.

import bass_rust as _bass_rust
from concourse.hw_specs import get_activation_tables


class _Bacc(bacc.Bacc):
    """Bacc that steers Exp and Ln to the combined natural_log_exp table set.

    The default greedy chooser puts Exp in exp_and_others and Ln in
    natural_log, forcing ACT table reloads per attention head. Shrinking the
    advertised contents of the single-function sets (ids stay canonical)
    makes both functions resolve to the set that has both."""

    def insert_act_table_loads(self):
        has_activation = any(
            isinstance(i, mybir.InstActivation)
            for b in self.main_func.blocks
            for i in b.instructions
        )
        if not has_activation:
            return
        tables = []
        for name, fns in get_activation_tables(self.m.arch).items():
            fns = set(fns)
            if name != "natural_log_exp_and_others":
                fns.discard(mybir.ActivationFunctionType.Exp)
                fns.discard(mybir.ActivationFunctionType.Ln)
            tables.append((name, fns))
        _bass_rust.insert_act_table_loads(self, tables)


BF16 = mybir.dt.bfloat16
F32 = mybir.dt.float32
NPBF16 = ml_dtypes.bfloat16
AF = mybir.ActivationFunctionType

S, DIM, H, D, DH = 8192, 1280, 16, 80, 40
DP = 96  # padded head dim for q/k channel grid (3 x 32)
NCORES = 8
L = S // NCORES  # 1024 tokens per core (= segment length)
CQK = 2 * H * D  # 2560 q+k channels (unpadded: DMA scatter has no
# partition-alignment rules, so the 96-grid padding is unnecessary)
N_CCH = DIM // 128  # 10 contraction chunks
N_PAIR = CQK // 256  # 10 qk column pairs
VA = DP + 1  # 97 cols per head in the PV operand (80 v + 16 pad + ones)
VAW = H * VA  # 1552
# stream order: q-pair j then k-pair j (q pairs 0..4, k pairs 5..9)
STREAM_ORDER = [p for j in range(5) for p in (j, 5 + j)]
# matmul outputs are capped at one PSUM bank (512 fp32) by the ISA
# (s3d3_mm_num_elements); activations/copies still read the full 2-bank tile
WIDE = False
# v/proj output-column sections, PSUM-bank-aligned so each (K-chunk, section)
# is a single matmul (one Ldweights each)
VSEC = ((0, 512), (512, 512), (1024, 256))


def _win(start):
    """Max legal partition span from a 32-aligned start (HW quadrant rule)."""
    s = start % 128
    return {0: 128, 32: 32, 64: 64, 96: 32}[s]


def _copy_pieces(src0, dst0, span):
    """Split a partition-range copy so both src and dst obey quadrant rules.
    Yields (src, dst, take)."""
    while span > 0:
        take = min(span, _win(src0), _win(dst0))
        yield src0, dst0, take
        src0 += take
        dst0 += take
        span -= take


def _chunk_heads(chunk):
    """Intersect 128-row chunk `chunk` of the 96-grid. Yields
    (row_in_chunk, grid_head, d0, span)."""
    r = 0
    while r < 128:
        g = 128 * chunk + r
        hh, d = g // DP, g % DP
        span = min(128 - r, DP - d)
        yield r, hh, d, span
        r += span


def _mm_cols(nc, out_ap_fn, lhsT, rhs_fn, width, start, stop):
    """Emit matmul(s) covering `width` output columns. With WIDE, one
    instruction; else 512-wide bank-sized pieces."""
    step = width if WIDE else min(512, width)
    c0 = 0
    while c0 < width:
        w = min(step, width - c0)
        nc.tensor.matmul(
            out_ap_fn(c0, w), lhsT=lhsT, rhs=rhs_fn(c0, w), start=start, stop=stop
        )
        c0 += w


class _Pump:
    """Attention micro-step emitter, fed between qkv chunks.

    Each head has a `pre` generator (rope + first scores — no ctx-PSUM use)
    and a `main` generator (sc/pv pipeline + finalize, which owns the single
    ctx PSUM slot). The pump round-robins the current head's main with the
    NEXT head's pre, so a finalize DVE burst never convoys the next head's
    rope, while PV accumulation stays strictly head-serial."""

    def __init__(self):
        self.backlog = []  # [pre, main] pairs not yet started
        self.cur = None  # [pre, main] of current head (owns ctx PSUM)
        self.nxt = None  # [pre, main] of next head (only pre may step)
        self.rr = 0

    def add(self, pre, main):
        self.backlog.append([pre, main])

    def _step(self, slot):
        """Advance slot's pre if any, else its main. Returns 'stepped',
        'pre_done' or 'done'."""
        gen = slot[0] if slot[0] is not None else slot[1]
        try:
            next(gen)
            return "stepped"
        except StopIteration:
            if slot[0] is not None:
                slot[0] = None
                return "pre_done"
            return "done"

    def run(self, max_steps):
        n = 0
        while n < max_steps:
            if self.cur is None:
                self.cur, self.nxt = self.nxt, None
            if self.cur is None and self.backlog:
                self.cur = self.backlog.pop(0)
            if self.nxt is None and self.backlog:
                self.nxt = self.backlog.pop(0)
            if self.cur is None:
                return n
            pick_nxt = (
                self.nxt is not None and self.nxt[0] is not None and self.rr % 4 == 3
            )
            self.rr += 1
            slot = self.nxt if pick_nxt else self.cur
            r = self._step(slot)
            if r == "stepped":
                n += 1
            elif r == "done":
                self.cur = None
        return n

    def drain(self):
        while self.run(1 << 30):
            pass


def _build_body(ctx: ExitStack, tc: tile.TileContext, io, with_bias):
    nc = tc.nc
    hTd, vwd, wpd, qkwd = io["hT"], io["vw"], io["wp"], io["qkw"]
    rotd, out = io["rot"], io["out"]
    bqk, bv, bp = io["bqk"], io["bv"], io["bp"]

    # ---- pools ----
    stat = ctx.enter_context(tc.tile_pool(name="stat", bufs=1))
    wbig = ctx.enter_context(tc.tile_pool(name="wbig", bufs=1))
    qk_p = ctx.enter_context(tc.tile_pool(name="qkp", bufs=10))
    pair_p = ctx.enter_context(tc.tile_pool(name="pairp", bufs=3))
    stg_p = ctx.enter_context(tc.tile_pool(name="stgp", bufs=3))
    es_p = ctx.enter_context(tc.tile_pool(name="esp", bufs=7))
    y_p = ctx.enter_context(tc.tile_pool(name="yp", bufs=4))
    ct_p = ctx.enter_context(tc.tile_pool(name="ctp", bufs=3))
    rbb_p = ctx.enter_context(tc.tile_pool(name="rbbp", bufs=2))
    rc_p = ctx.enter_context(tc.tile_pool(name="rcp", bufs=2))
    out_p = ctx.enter_context(tc.tile_pool(name="outp", bufs=2))
    # one shared 3-slot PSUM pool (6 banks): qkv chunks, scores, rope
    # shuffles, v groups and proj all cycle through it — every slot reuse is
    # >=3 allocations (several us) away, so nothing stalls on PSUM frees.
    ps_main = ctx.enter_context(tc.tile_pool(name="psmain", bufs=3, space="PSUM"))
    ps_ctx = ctx.enter_context(tc.tile_pool(name="psctx", bufs=1, space="PSUM"))

    def static(shape, dtype, name):
        return stat.tile(shape, dtype, name=name, tag=name)

    # ---- static tiles ----
    hT = [static([128, 5 * L], BF16, f"hT{i}") for i in range(2)]
    # vw/wp stream through shared single-slot-per-section pools: each proj
    # weight section reuses its v section's slot once the v phase is done
    vw = [
        wbig.tile([128, N_CCH * ws], BF16, tag=f"wsec{i}", name=f"vw{i}")
        for i, (o, ws) in enumerate(VSEC)
    ]
    _qk_tiles = {}

    def qk_sb(h):
        if h not in _qk_tiles:
            t_ = qk_p.tile([128, 2 * L], BF16, tag="qkp", name=f"qk{h}")
            if h < 10:  # first pass through the ring: zero the pad rows the
                # scatter never touches (they stay zero on slot reuse).
                # Engine access must start at a 64-boundary; rows [64:80)
                # are re-written by the scatter DMAs afterwards.
                nc.gpsimd.memset(t_[64:128, :], 0.0)
            _qk_tiles[h] = t_
        return _qk_tiles[h]

    vaug = [static([128, VAW], BF16, f"vaug{t}") for t in range(8)]
    ctxn = [static([128, L], BF16, f"ctxn{c}") for c in range(N_CCH)]
    s96 = static([DP, L], BF16, "s96")
    c96 = static([DP, L], BF16, "c96")

    def hT_ap(c, tok0, ntok):
        """hT chunk c, token slice [tok0, tok0+ntok)."""
        t_ = hT[c // 5]
        return t_[:, L * (c % 5) + tok0 : L * (c % 5) + tok0 + ntok]

    # ---- constants / prologue DMAs ----
    # hT + the first v weight section lead: the v phase's first matmul is the
    # earliest possible PE work. Each of hT[0]/vw[0] is split into partition
    # halves on separate issue queues so DGE descriptor generation (~50ns
    # per partition-row descriptor, the binding rate for these loads) runs
    # 4-wide in parallel.
    nc.sync.dma_start(hT[0][0:64, :], hTd[0:64, :])
    nc.scalar.dma_start(hT[0][64:128, :], hTd[64:128, :])
    nc.gpsimd.dma_start(vw[0][0:64, :], vwd[0:64, 0 : N_CCH * VSEC[0][1]])
    nc.gpsimd.dma_start(vw[0][64:128, :], vwd[64:128, 0 : N_CCH * VSEC[0][1]])
    nc.sync.dma_start(hT[1][0:64, :], hTd[128:192, :])
    nc.scalar.dma_start(hT[1][64:128, :], hTd[192:256, :])
    for i in range(1, 3):
        nc.gpsimd.dma_start(
            vw[i][:], vwd[128 * i : 128 * (i + 1), 0 : N_CCH * VSEC[i][1]]
        )
    # sin/cos precomputed on host ([2*96, L]: s96 then c96); rotate_half's
    # sign is folded into s96 rows [0:40) (see host_prep)
    nc.sync.dma_start(s96[:], rotd[0:DP, :])
    nc.sync.dma_start(c96[:], rotd[DP : 2 * DP, :])
    ones_att = static([1, DP], BF16, "ones_att")
    nc.gpsimd.memset(ones_att[:], 1.0)

    if with_bias:
        bqk_sb = static([1, CQK], BF16, "bqk_sb")
        nc.sync.dma_start(bqk_sb[:], bqk[:, :])
        bv_sb = static([1, DIM], BF16, "bv_sb")
        nc.sync.dma_start(bv_sb[:], bv[:, :])
        bp_sb = static([1, DIM], BF16, "bp_sb")
        nc.sync.dma_start(bp_sb[:], bp[:, :])
        ones_row = static([1, L], BF16, "ones_row")
        nc.vector.memset(ones_row[:], 1.0)

    pair_tiles = {}

    def fetch_pair(idx):
        if idx < len(STREAM_ORDER):
            pr = STREAM_ORDER[idx]
            w = pair_p.tile([128, 256 * N_CCH], BF16, tag="pairp", name=f"pw{pr}")
            nc.gpsimd.dma_start(w[:], qkwd[128 * pr : 128 * (pr + 1), :])
            pair_tiles[pr] = w

    # dummy exp: pulls the Exp/Ln ACT-table load into the idle prologue
    dume = rc_p.tile([1, 16], F32, tag="dume", name="dume")
    nc.scalar.activation(dume[:], dume[:], AF.Exp)

    # vaug pad + ones columns (v copies never touch them)
    for t in range(8):
        va3 = vaug[t][:].rearrange("p (h e) -> p h e", h=H)
        nc.gpsimd.memset(va3[:, :, D:DP], 0.0)
        nc.gpsimd.memset(va3[:, :, DP : DP + 1], 1.0)

    # ---- v phase: natural layout, scattered into vaug ----
    # section s covers v out channels [o, o+ws): one PSUM-bank-sized matmul
    # per K-chunk (1 Ldweights apiece)
    def v_group(t, s):
        o, ws = VSEC[s]
        ps = ps_main.tile([128, L], F32, tag="psmain", name="vps")
        for c in range(N_CCH):
            nc.tensor.matmul(
                ps[:, 0:ws],
                lhsT=hT_ap(c, 128 * t, 128),
                rhs=vw[s][:, ws * c : ws * c + ws],
                start=(c == 0),
                stop=(c == N_CCH - 1 and not with_bias),
            )
        if with_bias:
            nc.tensor.matmul(
                ps[:, 0:ws],
                lhsT=ones_row[0:1, 0:128],
                rhs=bv_sb[0:1, o : o + ws],
                start=False,
                stop=True,
            )
        # scatter psum v columns into vaug head slots (DVE copies — Vector
        # is otherwise idle during the v phase, and ScalarE must stay clear
        # for the exp chain later)
        va3 = vaug[t][:].rearrange("p (h e) -> p h e", h=H)
        c0 = 0
        while c0 < ws:
            ch = o + c0
            h, dd = ch // D, ch % D
            if dd == 0 and ws - c0 >= D:
                nh = (ws - c0) // D  # whole heads in one strided copy
                nc.vector.tensor_copy(
                    va3[:, h : h + nh, 0:D],
                    ps[:, c0 : c0 + nh * D].rearrange("p (h e) -> p h e", e=D),
                )
                c0 += nh * D
            else:
                take = min(D - dd, ws - c0)
                nc.vector.tensor_copy(
                    va3[:, h, dd : dd + take], ps[:, c0 : c0 + take]
                )
                c0 += take

    for s in range(3):  # s outer: section s only needs vw[s]'s DMA
        for t in range(8):
            v_group(t, s)
            if s == 0 and t < 3:
                # first qk pair fetches AFTER the critical hT/vw DMAs — the
                # shared DMA engines process transfers in issue order, and
                # pairs aren't consumed until the v phase finishes anyway
                fetch_pair(t)

    # proj weights into the freed v-weight slots (reads happen at proj time)
    wp = [
        wbig.tile([128, N_CCH * ws], BF16, tag=f"wsec{i}", name=f"wp{i}")
        for i, (o, ws) in enumerate(VSEC)
    ]
    for i in range(3):
        nc.sync.dma_start(wp[i][:], wpd[128 * i : 128 * (i + 1), 0 : N_CCH * VSEC[i][1]])

    # ---- attention micro-step machinery ----
    rc_dram = nc.dram_tensor("rcd", [H, L], BF16).ap()
    ctx_parts = []  # (chunk, dst0, head, src0, take) ctxn assembly pieces
    for c in range(N_CCH):
        r = 0
        while r < 128:
            g = 128 * c + r
            h, d = g // D, g % D
            take = min(128 - r, D - d)
            ctx_parts.append((c, r, h, d, take))
            r += take
    pending_tails = []

    def flush_tail():
        if pending_tails:
            pending_tails.pop(0)()

    def make_head(h):
        es_tiles = [None] * 8

        def sc_step(t):
            scp = ps_main.tile([128, L], F32, tag="psmain", name="scps")
            _mm_cols(
                nc,
                lambda c0, w, scp=scp: scp[:, c0 : c0 + w],
                qk_sb(h)[:, L + 128 * t : L + 128 * (t + 1)],
                lambda c0, w: qk_sb(h)[:, c0 : c0 + w],
                L,
                True,
                True,
            )
            e = es_p.tile([128, L], BF16, tag="esp", name="est")
            nc.scalar.activation(e[:], scp[:], AF.Exp)
            es_tiles[t] = e

        def pre():
            # rope: x = x*cos + swap(x)*ssin, per q/k part. swap (the
            # rotate_half permutation) is an SBUF->SBUF DMA (no partition
            # alignment rules); the rotate sign lives in ssin = s96
            # (rows [0:40) = -sin, [40:80) = +sin).
            for part in range(2):
                cs = slice(L * part, L * (part + 1))
                xh = qk_sb(h)[0 : 2 * DH, cs]
                sw = y_p.tile([DP, L], BF16, tag="yp", name="ropesw")
                deng = nc.sync if (h + part) % 2 == 0 else nc.gpsimd
                deng.dma_start(sw[0:DH, :], qk_sb(h)[DH : 2 * DH, cs])
                deng.dma_start(sw[DH : 2 * DH, :], qk_sb(h)[0:DH, cs])
                y = y_p.tile([DP, L], BF16, tag="yp", name="ropey")
                nc.vector.tensor_mul(y[0 : 2 * DH, :], sw[0 : 2 * DH, :], s96[0 : 2 * DH, :])
                y2 = y_p.tile([DP, L], BF16, tag="yp", name="ropey2")
                nc.vector.tensor_mul(y2[0 : 2 * DH, :], xh, c96[0 : 2 * DH, :])
                nc.vector.tensor_add(xh, y2[0 : 2 * DH, :], y[0 : 2 * DH, :])
                yield
            sc_step(0)
            yield

        def main():
            ctx_ps = ps_ctx.tile([VA, L], F32, tag="psctx", name="ctxps")

            def pv_step(t):
                for half in range(2):
                    nc.tensor.matmul(
                        ctx_ps[:, 512 * half : 512 * (half + 1)],
                        lhsT=vaug[t][:, VA * h : VA * (h + 1)],
                        rhs=es_tiles[t][:, 512 * half : 512 * (half + 1)],
                        start=(t == 0),
                        stop=(t == 7),
                    )
                es_tiles[t] = None

            # scores lead pv by 3 so pv(0) never waits on the previous
            # head's ctx-slot release (reciprocal/copy) or exp latency
            sc_step(1)
            yield
            sc_step(2)
            yield
            for t in range(3, 8):
                sc_step(t)
                pv_step(t - 3)
                yield
            pv_step(5)
            yield
            pv_step(6)
            pv_step(7)
            # finalize: reciprocal of denominator (row 96) on DVE in parallel
            # with the unnormalized ctx copy on ScalarE (frees the ctx PSUM
            # slot for the next head ~1.2us sooner)
            # 1/den = exp(-ln(den)) on ScalarE: DVE's reciprocal runs ~6.5us
            # per [1,1024] row on real HW; ln+exp are ~1.1us each and both
            # live in the natural_log_exp table set (see _Bacc)
            rc = rc_p.tile([1, L], BF16, tag="rcp3", name="recip")
            lt = rc_p.tile([1, L], F32, tag="lnt", name="lnt")
            nc.scalar.activation(lt[:], ctx_ps[DP : DP + 1, :], AF.Ln)
            nc.scalar.activation(rc[:], lt[:], AF.Exp, scale=-1.0)
            ct = ct_p.tile([D, L], BF16, tag="ctp", name="ctn")
            nc.vector.tensor_copy(ct[:], ctx_ps[0:D, :])
            if h >= H - 2:
                # tail heads: broadcast 1/den on the PE (ones outer product)
                # and finish immediately — the DRAM bounce latency would sit
                # on the critical path into proj
                rbb_ps = ps_main.tile([128, L], F32, tag="psmain", name="rbbps")
                _mm_cols(
                    nc,
                    lambda c0, w, rbb_ps=rbb_ps: rbb_ps[0:D, c0 : c0 + w],
                    ones_att[0:1, 0:D],
                    lambda c0, w: rc[0:1, c0 : c0 + w],
                    L,
                    True,
                    True,
                )
                nc.vector.tensor_mul(ct[:], ct[:], rbb_ps[0:D, :])
                for c, dst0, hh, src0, take in ctx_parts:
                    if hh == h:
                        nc.sync.dma_start(
                            ctxn[c][dst0 : dst0 + take, :], ct[src0 : src0 + take, :]
                        )
                yield
                return
            # broadcast 1/den across 80 rows via DRAM stride-0 read
            nc.sync.dma_start(rc_dram[h : h + 1, :], rc[:])
            rbb = rbb_p.tile([D, L], BF16, tag="rbbp", name="rbb")
            rcb = bass.AP(
                tensor=rc_dram.tensor,
                offset=rc_dram.offset + h * L,
                ap=[[0, D], [1, L]],
            )
            nc.sync.dma_start(rbb[:], rcb)

            def tail(h=h, ct=ct, rbb=rbb):
                nc.vector.tensor_mul(ct[:], ct[:], rbb[:])
                for c, dst0, hh, src0, take in ctx_parts:
                    if hh == h:
                        nc.sync.dma_start(
                            ctxn[c][dst0 : dst0 + take, :], ct[src0 : src0 + take, :]
                        )

            pending_tails.append(tail)
            if len(pending_tails) > 1:
                flush_tail()
            yield

        return pre(), main()

    pump = _Pump()

    # ---- qk streaming with attention pump ----
    scatter_n = 0
    ready = 0

    def out_chunk(pr, sub, act_only=False):
        nonlocal scatter_n
        f = 2 * pr + sub
        w = pair_tiles[pr]
        ps = ps_main.tile([128, L], F32, tag="psmain", name="qkps")
        for c in range(N_CCH):
            _mm_cols(
                nc,
                lambda c0, wd, ps=ps: ps[:, c0 : c0 + wd],
                w[:, 256 * c + 128 * sub : 256 * c + 128 * sub + 128],
                lambda c0, wd, c_=c: hT_ap(c_, c0, wd),
                L,
                c == 0,
                c == N_CCH - 1 and not with_bias,
            )
        if with_bias:
            nc.tensor.matmul(
                ps[:, 0:L],
                lhsT=bqk_sb[0:1, 128 * f : 128 * (f + 1)],
                rhs=ones_row[0:1, :],
                start=False,
                stop=True,
            )
        # one full-width staging copy (engine), then per-head-piece
        # SBUF->SBUF DMAs: DMA placement has no quadrant rules, so the
        # unpadded 80-grid scatters directly. Staging on DVE keeps the
        # ScalarE queue free for exp.
        stg = stg_p.tile([128, L], BF16, tag="stg", name="stg")
        if act_only:
            nc.scalar.activation(stg[:], ps[:], AF.Copy)
        else:
            nc.vector.tensor_copy(stg[:], ps[:])
        scatter_n += 1
        r = 0
        while r < 128:
            g = 128 * f + r
            hh, d0 = g // D, g % D
            take = min(128 - r, D - d0)
            is_k, h = hh // H, hh % H
            dma_eng = nc.sync if scatter_n % 2 == 0 else nc.scalar
            dma_eng.dma_start(
                qk_sb(h)[d0 : d0 + take, L * is_k : L * (is_k + 1)],
                stg[r : r + take, :],
            )
            r += take

    pending_heads = []
    for step, pr in enumerate(STREAM_ORDER):
        fetch_pair(step + 3)
        # heads readied at the previous step join now (one-step lag so the
        # first rope never waits on the scatter copies just emitted)
        for h in pending_heads:
            pump.add(*make_head(h))
        pending_heads = []
        for sub in range(2):
            # first two pairs: exp hasn't started, ScalarE is idle — route
            # all scatter copies there so rope never queues behind them on DVE
            out_chunk(pr, sub, act_only=step < 2)
            # ramp the pump: early heads' rope floods DVE right when the
            # scatter burst peaks, so start slow and catch up later
            pump.run(3 if step < 3 else (6 if step < 5 else 12))
        if pr >= 5:  # finished k-pair j
            j = pr - 5
            new_ready = min(H, (256 * (j + 1)) // D)
            pending_heads = list(range(ready, new_ready))
            ready = new_ready
    for h in pending_heads:
        pump.add(*make_head(h))
    pump.drain()
    for t in list(pending_tails):
        flush_tail()

    # ---- proj: out[t, :] = ctx_norm[t, :] @ wp ----
    for t in range(8):
        for s, (o, ws) in enumerate(VSEC):
            ot = out_p.tile([128, 512], F32, tag="outp", name="outt")
            pp = ps_main.tile([128, L], F32, tag="psmain", name="pjps")
            for c in range(N_CCH):
                nc.tensor.matmul(
                    pp[:, 0:ws],
                    lhsT=ctxn[c][:, 128 * t : 128 * (t + 1)],
                    rhs=wp[s][:, ws * c : ws * c + ws],
                    start=(c == 0),
                    stop=(c == N_CCH - 1 and not with_bias),
                )
            if with_bias:
                nc.tensor.matmul(
                    pp[:, 0:ws],
                    lhsT=ones_row[0:1, 0:128],
                    rhs=bp_sb[0:1, o : o + ws],
                    start=False,
                    stop=True,
                )
            # copy engine + out-DMA issue queue alternate so neither the
            # psum drain nor the final DMA serializes the tail
            if (3 * t + s) % 2 == 0:
                nc.vector.tensor_copy(ot[:, 0:ws], pp[:, 0:ws])
            else:
                nc.scalar.activation(ot[:, 0:ws], pp[:, 0:ws], AF.Copy)
            qeng = (nc.sync, nc.gpsimd, nc.scalar)[(3 * t + s) % 3]
            qeng.dma_start(
                out[128 * t : 128 * (t + 1), o : o + ws], ot[:, 0:ws]
            )


def build_nc(with_bias=False):
    nc = _Bacc("TRN2", target_bir_lowering=False, debug=False)
    io = {
        "hT": nc.dram_tensor("hT", [256, 5 * L], BF16, kind="ExternalInput").ap(),
        "vw": nc.dram_tensor("vw", [384, N_CCH * 512], BF16, kind="ExternalInput").ap(),
        "wp": nc.dram_tensor("wp", [384, N_CCH * 512], BF16, kind="ExternalInput").ap(),
        "qkw": nc.dram_tensor(
            "qkw", [N_PAIR * 128, 256 * N_CCH], BF16, kind="ExternalInput"
        ).ap(),
        "rot": nc.dram_tensor("rot", [2 * DP, L], BF16, kind="ExternalInput").ap(),
        "bqk": nc.dram_tensor("bqk", [1, CQK], BF16, kind="ExternalInput").ap(),
        "bv": nc.dram_tensor("bv", [1, DIM], BF16, kind="ExternalInput").ap(),
        "bp": nc.dram_tensor("bp", [1, DIM], BF16, kind="ExternalInput").ap(),
        "out": nc.dram_tensor("out", [L, DIM], F32, kind="ExternalOutput").ap(),
    }
    with tile.TileContext(nc) as tc:
        with ExitStack() as ctx:
            _build_body(ctx, tc, io, with_bias)
    nc.compile()
    return nc


def host_prep(inputs):
    """Host-side sharding + layout/dtype prep. Returns per-core in_maps."""
    h = np.asarray(inputs["hidden_states"], np.float32)
    rot = np.asarray(inputs["rotary_pos_emb"], np.float32)
    wqkv = np.asarray(inputs["w_qkv"], np.float32)
    bqkv = np.asarray(inputs["b_qkv"], np.float32)
    wpf = np.asarray(inputs["w_proj"], np.float32)
    bpf = np.asarray(inputs["b_proj"], np.float32)

    scale = float(D) ** -0.5
    # unpadded q/k weight rows (q first, scale folded in), packed
    # [pair, p, c, j] = wqk[256*pair + j, 128*c + p]
    wqk = np.concatenate([wqkv[:DIM] * scale, wqkv[DIM : 2 * DIM]], axis=0)
    bqk96 = np.concatenate([bqkv[:DIM] * scale, bqkv[DIM : 2 * DIM]])[None, :]
    qkw = (
        wqk.reshape(N_PAIR, 256, N_CCH, 128)
        .transpose(0, 3, 2, 1)
        .reshape(N_PAIR * 128, 256 * N_CCH)
    )
    def pack_sections(wT):
        # wT [in 1280, out 1280] -> [3*128, 10*512]: section s rows p hold
        # [c, outcols o:o+ws] at in-row 128c+p (short sections zero-padded)
        outp = np.zeros((3 * 128, N_CCH * 512), np.float32)
        w3 = wT.reshape(N_CCH, 128, DIM)  # [c, p, j]
        for s, (o, ws) in enumerate(VSEC):
            sec = w3[:, :, o : o + ws].transpose(1, 0, 2)  # [p, c, ws]
            outp[128 * s : 128 * (s + 1), 0 : N_CCH * ws] = sec.reshape(
                128, N_CCH * ws
            )
        return outp

    vw = pack_sections(np.ascontiguousarray(wqkv[2 * DIM :].T))
    wpp = pack_sections(np.ascontiguousarray(wpf.T))

    base = {
        "qkw": qkw.astype(NPBF16),
        "vw": vw.astype(NPBF16),
        "wp": wpp.astype(NPBF16),
        "bqk": bqk96.astype(NPBF16),
        "bv": bqkv[None, 2 * DIM :].astype(NPBF16),
        "bp": bpf[None, :].astype(NPBF16),
    }
    hT_full = np.ascontiguousarray(h.T)  # [1280, 8192]
    # host-side sin/cos on the padded 96-row head grid. rotate_half's sign
    # is folded into the sin rows: rope(x)[0:40] = x[0:40]*cos - x[40:80]*sin
    # and rope(x)[40:80] = x[40:80]*cos + x[0:40]*sin, so with the kernel's
    # swapped operand sw = concat(x[40:80], x[0:40]) the sin table is
    # [-sin; +sin].
    rotT = rot.T  # [40, 8192]
    sincos = np.zeros((2 * DP, S), np.float32)
    sincos[0:DH] = -np.sin(rotT)
    sincos[DH : 2 * DH] = np.sin(rotT)
    sincos[DP : DP + DH] = sincos[DP + DH : DP + 2 * DH] = np.cos(rotT)
    in_maps = []
    for cc in range(NCORES):
        sl = slice(L * cc, L * (cc + 1))
        hTc = hT_full[:, sl]  # [1280, 1024]
        # [half, p, c(5), tok]
        hp = (
            hTc.reshape(2, 5, 128, L)
            .transpose(0, 2, 1, 3)
            .reshape(256, 5 * L)
        )
        m = dict(base)
        m["hT"] = np.ascontiguousarray(hp).astype(NPBF16)
        m["rot"] = np.ascontiguousarray(sincos[:, sl]).astype(NPBF16)
        in_maps.append(m)
    return in_maps


_NC = {}


def _get_nc(with_bias=False):
    if with_bias not in _NC:
        _NC[with_bias] = build_nc(with_bias)
    return _NC[with_bias]


def run(inputs, trace=False, trace_kwargs=None):
    from concourse.bass_utils import run_bass_kernel_spmd

    with_bias = bool(
        np.any(np.asarray(inputs["b_qkv"])) or np.any(np.asarray(inputs["b_proj"]))
    )
    nc = _get_nc(with_bias)
    in_maps = host_prep(inputs)
    kw = {}
    if trace:
        kw = dict(trace=True, **(trace_kwargs or {}))
        kw.setdefault("trace_cores", list(range(NCORES)))
    res = run_bass_kernel_spmd(nc, in_maps, list(range(NCORES)), **kw)
    outs = np.concatenate([res.results[i]["out"] for i in range(NCORES)], axis=0)
    return outs.astype(np.float32), res


def kernel(**inputs) -> np.ndarray:
    out, _ = run(inputs)
    return out



# revision 19
# speedup vs baseline: 1.0495x; 1.0495x over previous
"""DotsVisionAttention Trainium2 kernel (v1 — pipelined rewrite).

Full-input contract: kernel(**inputs) takes the unsharded tensors from
setup_inputs() and returns the full [8192, 1280] fp32 output.

Sharding: data-parallel over the 8 packed image segments (attention is
block-diagonal with 8 equal segments of 1024 tokens) — core i processes
tokens [1024*i, 1024*(i+1)), no collectives.

Differences vs v0 baseline:
  * All weight/activation DMAs are host-packed so each transfer is one
    big partition-major descriptor block (128 descriptors) — Pool/SP
    descriptor-generation time drops ~10x.
  * v is produced in natural [token, ch] layout (no PE transposes) and
    scattered into ones-augmented per-head PV operand tiles.
  * Scores matmul emits a single 1024-wide instruction into a 2-bank
    PSUM tile; exp processes 1024 columns per ScalarE instruction.
  * Softmax reciprocal on DVE (vector.reciprocal); denominator taken
    from PSUM row 96 of the PV accumulator (ones column trick).
  * proj contracts over the unpadded 1280 ctx channels (10 K-chunks).
  * Attention for head h is emitted as micro-steps interleaved between
    qkv chunk matmul groups ("pump") so the PE never stalls on the
    ScalarE exp chain and stays at max p-state.

v2 changes (this session):
  * rope's rotate_half is an SBUF->SBUF DMA swap (sign folded into the
    host-side sin table) instead of a PE permutation matmul — removes
    32k matmul columns (~14us of PE) and a psum slot per head-part.
  * prologue hT/vw0 loads split across 4 issue queues (partition
    halves) so descriptor generation runs in parallel.
  * psum->sbuf copies rebalanced: v-scatter + qk staging on DVE
    (idle during those phases), keeping ScalarE clear for the exp
    chain that gates the attention pipeline.
  * output DMAs spread across issue queues.
"""

from contextlib import ExitStack

import ml_dtypes
import numpy as np

import concourse.bass as bass
import concourse.bass_utils as _bass_utils
import concourse.tile as tile
from concourse import bacc, mybir


# NOTE: walrus's --enable-ldw-opt would dedupe the ~100ns Ldweights that
# precedes every matmul (about 170us of TensorE time here), but this build
# rejects bass-emitted Ldweights ("not compatible with LDW optimization"),
# so the only lever is emitting fewer matmuls.

import bass_rust as _bass_rust
from concourse.hw_specs import get_activation_tables


class _Bacc(bacc.Bacc):
    """Bacc that steers Exp and Ln to the combined natural_log_exp table set.

    The default greedy chooser puts Exp in exp_and_others and Ln in
    natural_log, forcing ACT table reloads per attention head. Shrinking the
    advertised contents of the single-function sets (ids stay canonical)
    makes both functions resolve to the set that has both."""

    def insert_act_table_loads(self):
        has_activation = any(
            isinstance(i, mybir.InstActivation)
            for b in self.main_func.blocks
            for i in b.instructions
        )
        if not has_activation:
            return
        tables = []
        for name, fns in get_activation_tables(self.m.arch).items():
            fns = set(fns)
            if name != "natural_log_exp_and_others":
                fns.discard(mybir.ActivationFunctionType.Exp)
                fns.discard(mybir.ActivationFunctionType.Ln)
            tables.append((name, fns))
        _bass_rust.insert_act_table_loads(self, tables)


BF16 = mybir.dt.bfloat16
F32 = mybir.dt.float32
NPBF16 = ml_dtypes.bfloat16
AF = mybir.ActivationFunctionType

S, DIM, H, D, DH = 8192, 1280, 16, 80, 40
DP = 96  # padded head dim for q/k channel grid (3 x 32)
NCORES = 8
L = S // NCORES  # 1024 tokens per core (= segment length)
CQK = 2 * H * D  # 2560 q+k channels (unpadded: DMA scatter has no
# partition-alignment rules, so the 96-grid padding is unnecessary)
N_CCH = DIM // 128  # 10 contraction chunks
N_PAIR = CQK // 256  # 10 qk column pairs
VA = DP + 1  # 97 cols per head in the PV operand (80 v + 16 pad + ones)
VAW = H * VA  # 1552
# stream order: q-pair j then k-pair j (q pairs 0..4, k pairs 5..9)
STREAM_ORDER = [p for j in range(5) for p in (j, 5 + j)]
# matmul outputs are capped at one PSUM bank (512 fp32) by the ISA
# (s3d3_mm_num_elements); activations/copies still read the full 2-bank tile
WIDE = False
# v/proj output-column sections, PSUM-bank-aligned so each (K-chunk, section)
# is a single matmul (one Ldweights each)
VSEC = ((0, 512), (512, 512), (1024, 256))


def _win(start):
    """Max legal partition span from a 32-aligned start (HW quadrant rule)."""
    s = start % 128
    return {0: 128, 32: 32, 64: 64, 96: 32}[s]


def _copy_pieces(src0, dst0, span):
    """Split a partition-range copy so both src and dst obey quadrant rules.
    Yields (src, dst, take)."""
    while span > 0:
        take = min(span, _win(src0), _win(dst0))
        yield src0, dst0, take
        src0 += take
        dst0 += take
        span -= take


def _chunk_heads(chunk):
    """Intersect 128-row chunk `chunk` of the 96-grid. Yields
    (row_in_chunk, grid_head, d0, span)."""
    r = 0
    while r < 128:
        g = 128 * chunk + r
        hh, d = g // DP, g % DP
        span = min(128 - r, DP - d)
        yield r, hh, d, span
        r += span


def _mm_cols(nc, out_ap_fn, lhsT, rhs_fn, width, start, stop):
    """Emit matmul(s) covering `width` output columns. With WIDE, one
    instruction; else 512-wide bank-sized pieces."""
    step = width if WIDE else min(512, width)
    c0 = 0
    while c0 < width:
        w = min(step, width - c0)
        nc.tensor.matmul(
            out_ap_fn(c0, w), lhsT=lhsT, rhs=rhs_fn(c0, w), start=start, stop=stop
        )
        c0 += w


class _Pump:
    """Attention micro-step emitter, fed between qkv chunks.

    Each head has a `pre` generator (rope + first scores — no ctx-PSUM use)
    and a `main` generator (sc/pv pipeline + finalize, which owns the single
    ctx PSUM slot). The pump round-robins the current head's main with the
    NEXT head's pre, so a finalize DVE burst never convoys the next head's
    rope, while PV accumulation stays strictly head-serial."""

    def __init__(self):
        self.backlog = []  # [pre, main] pairs not yet started
        self.cur = None  # [pre, main] of current head (owns ctx PSUM)
        self.nxt = None  # [pre, main] of next head (only pre may step)
        self.rr = 0

    def add(self, pre, main):
        self.backlog.append([pre, main])

    def _step(self, slot):
        """Advance slot's pre if any, else its main. Returns 'stepped',
        'pre_done' or 'done'."""
        gen = slot[0] if slot[0] is not None else slot[1]
        try:
            next(gen)
            return "stepped"
        except StopIteration:
            if slot[0] is not None:
                slot[0] = None
                return "pre_done"
            return "done"

    def run(self, max_steps):
        n = 0
        while n < max_steps:
            if self.cur is None:
                self.cur, self.nxt = self.nxt, None
            if self.cur is None and self.backlog:
                self.cur = self.backlog.pop(0)
            if self.nxt is None and self.backlog:
                self.nxt = self.backlog.pop(0)
            if self.cur is None:
                return n
            pick_nxt = (
                self.nxt is not None and self.nxt[0] is not None and self.rr % 4 == 3
            )
            self.rr += 1
            slot = self.nxt if pick_nxt else self.cur
            r = self._step(slot)
            if r == "stepped":
                n += 1
            elif r == "done":
                self.cur = None
        return n

    def drain(self):
        while self.run(1 << 30):
            pass


def _build_body(ctx: ExitStack, tc: tile.TileContext, io, with_bias):
    nc = tc.nc
    hTd, vwd, wpd, qkwd = io["hT"], io["vw"], io["wp"], io["qkw"]
    rotd, out = io["rot"], io["out"]
    bqk, bv, bp = io["bqk"], io["bv"], io["bp"]

    # ---- pools ----
    stat = ctx.enter_context(tc.tile_pool(name="stat", bufs=1))
    wbig = ctx.enter_context(tc.tile_pool(name="wbig", bufs=1))
    qk_p = ctx.enter_context(tc.tile_pool(name="qkp", bufs=10))
    pair_p = ctx.enter_context(tc.tile_pool(name="pairp", bufs=3))
    stg_p = ctx.enter_context(tc.tile_pool(name="stgp", bufs=3))
    es_p = ctx.enter_context(tc.tile_pool(name="esp", bufs=7))
    y_p = ctx.enter_context(tc.tile_pool(name="yp", bufs=4))
    ct_p = ctx.enter_context(tc.tile_pool(name="ctp", bufs=3))
    rbb_p = ctx.enter_context(tc.tile_pool(name="rbbp", bufs=2))
    rc_p = ctx.enter_context(tc.tile_pool(name="rcp", bufs=2))
    out_p = ctx.enter_context(tc.tile_pool(name="outp", bufs=2))
    # one shared 3-slot PSUM pool (6 banks): qkv chunks, scores, rope
    # shuffles, v groups and proj all cycle through it — every slot reuse is
    # >=3 allocations (several us) away, so nothing stalls on PSUM frees.
    ps_main = ctx.enter_context(tc.tile_pool(name="psmain", bufs=3, space="PSUM"))
    ps_ctx = ctx.enter_context(tc.tile_pool(name="psctx", bufs=1, space="PSUM"))

    def static(shape, dtype, name):
        return stat.tile(shape, dtype, name=name, tag=name)

    # ---- static tiles ----
    hT = [static([128, 5 * L], BF16, f"hT{i}") for i in range(2)]
    # vw/wp stream through shared single-slot-per-section pools: each proj
    # weight section reuses its v section's slot once the v phase is done
    vw = [
        wbig.tile([128, N_CCH * ws], BF16, tag=f"wsec{i}", name=f"vw{i}")
        for i, (o, ws) in enumerate(VSEC)
    ]
    _qk_tiles = {}

    def qk_sb(h):
        if h not in _qk_tiles:
            t_ = qk_p.tile([128, 2 * L], BF16, tag="qkp", name=f"qk{h}")
            if h < 10:  # first pass through the ring: zero the pad rows the
                # scatter never touches (they stay zero on slot reuse).
                # Engine access must start at a 64-boundary; rows [64:80)
                # are re-written by the scatter DMAs afterwards.
                nc.gpsimd.memset(t_[64:128, :], 0.0)
            _qk_tiles[h] = t_
        return _qk_tiles[h]

    vaug = [static([128, VAW], BF16, f"vaug{t}") for t in range(8)]
    ctxn = [static([128, L], BF16, f"ctxn{c}") for c in range(N_CCH)]
    s96 = static([DP, L], BF16, "s96")
    c96 = static([DP, L], BF16, "c96")

    def hT_ap(c, tok0, ntok):
        """hT chunk c, token slice [tok0, tok0+ntok)."""
        t_ = hT[c // 5]
        return t_[:, L * (c % 5) + tok0 : L * (c % 5) + tok0 + ntok]

    # ---- constants / prologue DMAs ----
    # hT + the first v weight section lead: the v phase's first matmul is the
    # earliest possible PE work. Each of hT[0]/vw[0] is split into partition
    # halves on separate issue queues so DGE descriptor generation (~50ns
    # per partition-row descriptor, the binding rate for these loads) runs
    # 4-wide in parallel.
    # sync carries hT, scalar carries vw — the two hw DGE queues generate
    # descriptors concurrently (the gpsimd sw-dge queue starts ~5us later
    # and is reserved for the qk pair prefetches, which aren't needed until
    # the stream phase).
    nc.sync.dma_start(hT[0][:], hTd[0:128, :])
    nc.scalar.dma_start(vw[0][:], vwd[0:128, 0 : N_CCH * VSEC[0][1]])
    nc.sync.dma_start(hT[1][:], hTd[128:256, :])
    for i in range(1, 3):
        nc.scalar.dma_start(
            vw[i][:], vwd[128 * i : 128 * (i + 1), 0 : N_CCH * VSEC[i][1]]
        )
    # sin/cos precomputed on host ([2*96, L]: s96 then c96); rotate_half's
    # sign is folded into s96 rows [0:40) (see host_prep)
    nc.sync.dma_start(s96[:], rotd[0:DP, :])
    nc.sync.dma_start(c96[:], rotd[DP : 2 * DP, :])
    ones_att = static([1, DP], BF16, "ones_att")
    nc.gpsimd.memset(ones_att[:], 1.0)

    if with_bias:
        bqk_sb = static([1, CQK], BF16, "bqk_sb")
        nc.sync.dma_start(bqk_sb[:], bqk[:, :])
        bv_sb = static([1, DIM], BF16, "bv_sb")
        nc.sync.dma_start(bv_sb[:], bv[:, :])
        bp_sb = static([1, DIM], BF16, "bp_sb")
        nc.sync.dma_start(bp_sb[:], bp[:, :])
        ones_row = static([1, L], BF16, "ones_row")
        nc.vector.memset(ones_row[:], 1.0)

    pair_tiles = {}

    def fetch_pair(idx):
        if idx < len(STREAM_ORDER):
            pr = STREAM_ORDER[idx]
            w = pair_p.tile([128, 256 * N_CCH], BF16, tag="pairp", name=f"pw{pr}")
            nc.gpsimd.dma_start(w[:], qkwd[128 * pr : 128 * (pr + 1), :])
            pair_tiles[pr] = w

    # dummy exp: pulls the Exp/Ln ACT-table load into the idle prologue
    dume = rc_p.tile([1, 16], F32, tag="dume", name="dume")
    nc.scalar.activation(dume[:], dume[:], AF.Exp)

    # vaug pad + ones columns (v copies never touch them)
    for t in range(8):
        va3 = vaug[t][:].rearrange("p (h e) -> p h e", h=H)
        nc.gpsimd.memset(va3[:, :, D:DP], 0.0)
        nc.gpsimd.memset(va3[:, :, DP : DP + 1], 1.0)

    # ---- v phase: natural layout, scattered into vaug ----
    # section s covers v out channels [o, o+ws): one PSUM-bank-sized matmul
    # per K-chunk (1 Ldweights apiece)
    def v_group(t, s):
        o, ws = VSEC[s]
        ps = ps_main.tile([128, L], F32, tag="psmain", name="vps")
        for c in range(N_CCH):
            nc.tensor.matmul(
                ps[:, 0:ws],
                lhsT=hT_ap(c, 128 * t, 128),
                rhs=vw[s][:, ws * c : ws * c + ws],
                start=(c == 0),
                stop=(c == N_CCH - 1 and not with_bias),
            )
        if with_bias:
            nc.tensor.matmul(
                ps[:, 0:ws],
                lhsT=ones_row[0:1, 0:128],
                rhs=bv_sb[0:1, o : o + ws],
                start=False,
                stop=True,
            )
        # scatter psum v columns into vaug head slots (DVE copies — Vector
        # is otherwise idle during the v phase, and ScalarE must stay clear
        # for the exp chain later)
        va3 = vaug[t][:].rearrange("p (h e) -> p h e", h=H)
        c0 = 0
        while c0 < ws:
            ch = o + c0
            h, dd = ch // D, ch % D
            if dd == 0 and ws - c0 >= D:
                nh = (ws - c0) // D  # whole heads in one strided copy
                nc.vector.tensor_copy(
                    va3[:, h : h + nh, 0:D],
                    ps[:, c0 : c0 + nh * D].rearrange("p (h e) -> p h e", e=D),
                )
                c0 += nh * D
            else:
                take = min(D - dd, ws - c0)
                nc.vector.tensor_copy(
                    va3[:, h, dd : dd + take], ps[:, c0 : c0 + take]
                )
                c0 += take

    for s in range(3):  # s outer: section s only needs vw[s]'s DMA
        for t in range(8):
            v_group(t, s)
            if s == 0 and t < 3:
                # first qk pair fetches AFTER the critical hT/vw DMAs — the
                # shared DMA engines process transfers in issue order, and
                # pairs aren't consumed until the v phase finishes anyway
                fetch_pair(t)

    # proj weights into the freed v-weight slots (reads happen at proj time)
    wp = [
        wbig.tile([128, N_CCH * ws], BF16, tag=f"wsec{i}", name=f"wp{i}")
        for i, (o, ws) in enumerate(VSEC)
    ]
    for i in range(3):
        nc.sync.dma_start(wp[i][:], wpd[128 * i : 128 * (i + 1), 0 : N_CCH * VSEC[i][1]])

    # ---- attention micro-step machinery ----
    rc_dram = nc.dram_tensor("rcd", [H, L], BF16).ap()
    ctx_parts = []  # (chunk, dst0, head, src0, take) ctxn assembly pieces
    for c in range(N_CCH):
        r = 0
        while r < 128:
            g = 128 * c + r
            h, d = g // D, g % D
            take = min(128 - r, D - d)
            ctx_parts.append((c, r, h, d, take))
            r += take
    pending_tails = []

    def flush_tail():
        if pending_tails:
            pending_tails.pop(0)()

    def make_head(h):
        es_tiles = [None] * 8

        def sc_step(t):
            scp = ps_main.tile([128, L], F32, tag="psmain", name="scps")
            _mm_cols(
                nc,
                lambda c0, w, scp=scp: scp[:, c0 : c0 + w],
                qk_sb(h)[:, L + 128 * t : L + 128 * (t + 1)],
                lambda c0, w: qk_sb(h)[:, c0 : c0 + w],
                L,
                True,
                True,
            )
            e = es_p.tile([128, L], BF16, tag="esp", name="est")
            nc.scalar.activation(e[:], scp[:], AF.Exp)
            es_tiles[t] = e

        def pre():
            # rope: x = x*cos + swap(x)*ssin, per q/k part. swap (the
            # rotate_half permutation) is an SBUF->SBUF DMA (no partition
            # alignment rules); the rotate sign lives in ssin = s96
            # (rows [0:40) = -sin, [40:80) = +sin).
            for part in range(2):
                cs = slice(L * part, L * (part + 1))
                xh = qk_sb(h)[0 : 2 * DH, cs]
                sw = y_p.tile([DP, L], BF16, tag="yp", name="ropesw")
                nc.sync.dma_start(sw[0:DH, :], qk_sb(h)[DH : 2 * DH, cs])
                nc.sync.dma_start(sw[DH : 2 * DH, :], qk_sb(h)[0:DH, cs])
                y = y_p.tile([DP, L], BF16, tag="yp", name="ropey")
                nc.vector.tensor_mul(y[0 : 2 * DH, :], sw[0 : 2 * DH, :], s96[0 : 2 * DH, :])
                y2 = y_p.tile([DP, L], BF16, tag="yp", name="ropey2")
                nc.vector.tensor_mul(y2[0 : 2 * DH, :], xh, c96[0 : 2 * DH, :])
                nc.vector.tensor_add(xh, y2[0 : 2 * DH, :], y[0 : 2 * DH, :])
                yield
            sc_step(0)
            yield

        def main():
            ctx_ps = ps_ctx.tile([VA, L], F32, tag="psctx", name="ctxps")

            def pv_step(t):
                for half in range(2):
                    nc.tensor.matmul(
                        ctx_ps[:, 512 * half : 512 * (half + 1)],
                        lhsT=vaug[t][:, VA * h : VA * (h + 1)],
                        rhs=es_tiles[t][:, 512 * half : 512 * (half + 1)],
                        start=(t == 0),
                        stop=(t == 7),
                    )
                es_tiles[t] = None

            # scores lead pv by 3 so pv(0) never waits on the previous
            # head's ctx-slot release (reciprocal/copy) or exp latency
            sc_step(1)
            yield
            sc_step(2)
            yield
            for t in range(3, 8):
                sc_step(t)
                pv_step(t - 3)
                yield
            pv_step(5)
            yield
            pv_step(6)
            pv_step(7)
            # finalize: reciprocal of denominator (row 96) on DVE in parallel
            # with the unnormalized ctx copy on ScalarE (frees the ctx PSUM
            # slot for the next head ~1.2us sooner)
            # 1/den = exp(-ln(den)) on ScalarE: DVE's reciprocal runs ~6.5us
            # per [1,1024] row on real HW; ln+exp are ~1.1us each and both
            # live in the natural_log_exp table set (see _Bacc)
            rc = rc_p.tile([1, L], BF16, tag="rcp3", name="recip")
            lt = rc_p.tile([1, L], F32, tag="lnt", name="lnt")
            nc.scalar.activation(lt[:], ctx_ps[DP : DP + 1, :], AF.Ln)
            nc.scalar.activation(rc[:], lt[:], AF.Exp, scale=-1.0)
            ct = ct_p.tile([D, L], BF16, tag="ctp", name="ctn")
            nc.vector.tensor_copy(ct[:], ctx_ps[0:D, :])
            if h >= H - 2:
                # tail heads: broadcast 1/den on the PE (ones outer product)
                # and finish immediately — the DRAM bounce latency would sit
                # on the critical path into proj
                rbb_ps = ps_main.tile([128, L], F32, tag="psmain", name="rbbps")
                _mm_cols(
                    nc,
                    lambda c0, w, rbb_ps=rbb_ps: rbb_ps[0:D, c0 : c0 + w],
                    ones_att[0:1, 0:D],
                    lambda c0, w: rc[0:1, c0 : c0 + w],
                    L,
                    True,
                    True,
                )
                nc.vector.tensor_mul(ct[:], ct[:], rbb_ps[0:D, :])
                for c, dst0, hh, src0, take in ctx_parts:
                    if hh == h:
                        nc.sync.dma_start(
                            ctxn[c][dst0 : dst0 + take, :], ct[src0 : src0 + take, :]
                        )
                yield
                return
            # broadcast 1/den across 80 rows via DRAM stride-0 read
            nc.sync.dma_start(rc_dram[h : h + 1, :], rc[:])
            rbb = rbb_p.tile([D, L], BF16, tag="rbbp", name="rbb")
            rcb = bass.AP(
                tensor=rc_dram.tensor,
                offset=rc_dram.offset + h * L,
                ap=[[0, D], [1, L]],
            )
            nc.sync.dma_start(rbb[:], rcb)

            def tail(h=h, ct=ct, rbb=rbb):
                nc.vector.tensor_mul(ct[:], ct[:], rbb[:])
                for c, dst0, hh, src0, take in ctx_parts:
                    if hh == h:
                        nc.sync.dma_start(
                            ctxn[c][dst0 : dst0 + take, :], ct[src0 : src0 + take, :]
                        )

            pending_tails.append(tail)
            if len(pending_tails) > 1:
                flush_tail()
            yield

        return pre(), main()

    pump = _Pump()

    # ---- qk streaming with attention pump ----
    scatter_n = 0
    ready = 0

    def out_chunk(pr, sub, act_only=False):
        nonlocal scatter_n
        f = 2 * pr + sub
        w = pair_tiles[pr]
        ps = ps_main.tile([128, L], F32, tag="psmain", name="qkps")
        for c in range(N_CCH):
            _mm_cols(
                nc,
                lambda c0, wd, ps=ps: ps[:, c0 : c0 + wd],
                w[:, 256 * c + 128 * sub : 256 * c + 128 * sub + 128],
                lambda c0, wd, c_=c: hT_ap(c_, c0, wd),
                L,
                c == 0,
                c == N_CCH - 1 and not with_bias,
            )
        if with_bias:
            nc.tensor.matmul(
                ps[:, 0:L],
                lhsT=bqk_sb[0:1, 128 * f : 128 * (f + 1)],
                rhs=ones_row[0:1, :],
                start=False,
                stop=True,
            )
        # one full-width staging copy (engine), then per-head-piece
        # SBUF->SBUF DMAs: DMA placement has no quadrant rules, so the
        # unpadded 80-grid scatters directly. Staging on DVE keeps the
        # ScalarE queue free for exp.
        stg = stg_p.tile([128, L], BF16, tag="stg", name="stg")
        if act_only:
            nc.scalar.activation(stg[:], ps[:], AF.Copy)
        else:
            nc.vector.tensor_copy(stg[:], ps[:])
        scatter_n += 1
        r = 0
        while r < 128:
            g = 128 * f + r
            hh, d0 = g // D, g % D
            take = min(128 - r, D - d0)
            is_k, h = hh // H, hh % H
            dma_eng = nc.sync if scatter_n % 2 == 0 else nc.scalar
            dma_eng.dma_start(
                qk_sb(h)[d0 : d0 + take, L * is_k : L * (is_k + 1)],
                stg[r : r + take, :],
            )
            r += take

    pending_heads = []
    for step, pr in enumerate(STREAM_ORDER):
        fetch_pair(step + 3)
        # heads readied at the previous step join now (one-step lag so the
        # first rope never waits on the scatter copies just emitted)
        for h in pending_heads:
            pump.add(*make_head(h))
        pending_heads = []
        for sub in range(2):
            # first two pairs: exp hasn't started, ScalarE is idle — route
            # all scatter copies there so rope never queues behind them on DVE
            out_chunk(pr, sub, act_only=step < 2)
            # ramp the pump: early heads' rope floods DVE right when the
            # scatter burst peaks, so start slow and catch up later
            pump.run(3 if step < 3 else (6 if step < 5 else 12))
        if pr >= 5:  # finished k-pair j
            j = pr - 5
            new_ready = min(H, (256 * (j + 1)) // D)
            pending_heads = list(range(ready, new_ready))
            ready = new_ready
    for h in pending_heads:
        pump.add(*make_head(h))
    pump.drain()
    for t in list(pending_tails):
        flush_tail()

    # ---- proj: out[t, :] = ctx_norm[t, :] @ wp ----
    for t in range(8):
        for s, (o, ws) in enumerate(VSEC):
            ot = out_p.tile([128, 512], F32, tag="outp", name="outt")
            pp = ps_main.tile([128, L], F32, tag="psmain", name="pjps")
            for c in range(N_CCH):
                nc.tensor.matmul(
                    pp[:, 0:ws],
                    lhsT=ctxn[c][:, 128 * t : 128 * (t + 1)],
                    rhs=wp[s][:, ws * c : ws * c + ws],
                    start=(c == 0),
                    stop=(c == N_CCH - 1 and not with_bias),
                )
            if with_bias:
                nc.tensor.matmul(
                    pp[:, 0:ws],
                    lhsT=ones_row[0:1, 0:128],
                    rhs=bp_sb[0:1, o : o + ws],
                    start=False,
                    stop=True,
                )
            # copy engine + out-DMA issue queue alternate so neither the
            # psum drain nor the final DMA serializes the tail
            if (3 * t + s) % 2 == 0:
                nc.vector.tensor_copy(ot[:, 0:ws], pp[:, 0:ws])
            else:
                nc.scalar.activation(ot[:, 0:ws], pp[:, 0:ws], AF.Copy)
            qeng = nc.sync if (3 * t + s) % 2 == 0 else nc.scalar
            qeng.dma_start(
                out[128 * t : 128 * (t + 1), o : o + ws], ot[:, 0:ws]
            )


def build_nc(with_bias=False):
    nc = _Bacc("TRN2", target_bir_lowering=False, debug=False)
    io = {
        "hT": nc.dram_tensor("hT", [256, 5 * L], BF16, kind="ExternalInput").ap(),
        "vw": nc.dram_tensor("vw", [384, N_CCH * 512], BF16, kind="ExternalInput").ap(),
        "wp": nc.dram_tensor("wp", [384, N_CCH * 512], BF16, kind="ExternalInput").ap(),
        "qkw": nc.dram_tensor(
            "qkw", [N_PAIR * 128, 256 * N_CCH], BF16, kind="ExternalInput"
        ).ap(),
        "rot": nc.dram_tensor("rot", [2 * DP, L], BF16, kind="ExternalInput").ap(),
        "bqk": nc.dram_tensor("bqk", [1, CQK], BF16, kind="ExternalInput").ap(),
        "bv": nc.dram_tensor("bv", [1, DIM], BF16, kind="ExternalInput").ap(),
        "bp": nc.dram_tensor("bp", [1, DIM], BF16, kind="ExternalInput").ap(),
        "out": nc.dram_tensor("out", [L, DIM], F32, kind="ExternalOutput").ap(),
    }
    with tile.TileContext(nc) as tc:
        with ExitStack() as ctx:
            _build_body(ctx, tc, io, with_bias)
    nc.compile()
    return nc


def host_prep(inputs):
    """Host-side sharding + layout/dtype prep. Returns per-core in_maps."""
    h = np.asarray(inputs["hidden_states"], np.float32)
    rot = np.asarray(inputs["rotary_pos_emb"], np.float32)
    wqkv = np.asarray(inputs["w_qkv"], np.float32)
    bqkv = np.asarray(inputs["b_qkv"], np.float32)
    wpf = np.asarray(inputs["w_proj"], np.float32)
    bpf = np.asarray(inputs["b_proj"], np.float32)

    scale = float(D) ** -0.5
    # unpadded q/k weight rows (q first, scale folded in), packed
    # [pair, p, c, j] = wqk[256*pair + j, 128*c + p]
    wqk = np.concatenate([wqkv[:DIM] * scale, wqkv[DIM : 2 * DIM]], axis=0)
    bqk96 = np.concatenate([bqkv[:DIM] * scale, bqkv[DIM : 2 * DIM]])[None, :]
    qkw = (
        wqk.reshape(N_PAIR, 256, N_CCH, 128)
        .transpose(0, 3, 2, 1)
        .reshape(N_PAIR * 128, 256 * N_CCH)
    )
    def pack_sections(wT):
        # wT [in 1280, out 1280] -> [3*128, 10*512]: section s rows p hold
        # [c, outcols o:o+ws] at in-row 128c+p (short sections zero-padded)
        outp = np.zeros((3 * 128, N_CCH * 512), np.float32)
        w3 = wT.reshape(N_CCH, 128, DIM)  # [c, p, j]
        for s, (o, ws) in enumerate(VSEC):
            sec = w3[:, :, o : o + ws].transpose(1, 0, 2)  # [p, c, ws]
            outp[128 * s : 128 * (s + 1), 0 : N_CCH * ws] = sec.reshape(
                128, N_CCH * ws
            )
        return outp

    vw = pack_sections(np.ascontiguousarray(wqkv[2 * DIM :].T))
    wpp = pack_sections(np.ascontiguousarray(wpf.T))

    base = {
        "qkw": qkw.astype(NPBF16),
        "vw": vw.astype(NPBF16),
        "wp": wpp.astype(NPBF16),
        "bqk": bqk96.astype(NPBF16),
        "bv": bqkv[None, 2 * DIM :].astype(NPBF16),
        "bp": bpf[None, :].astype(NPBF16),
    }
    hT_full = np.ascontiguousarray(h.T)  # [1280, 8192]
    # host-side sin/cos on the padded 96-row head grid. rotate_half's sign
    # is folded into the sin rows: rope(x)[0:40] = x[0:40]*cos - x[40:80]*sin
    # and rope(x)[40:80] = x[40:80]*cos + x[0:40]*sin, so with the kernel's
    # swapped operand sw = concat(x[40:80], x[0:40]) the sin table is
    # [-sin; +sin].
    rotT = rot.T  # [40, 8192]
    sincos = np.zeros((2 * DP, S), np.float32)
    sincos[0:DH] = -np.sin(rotT)
    sincos[DH : 2 * DH] = np.sin(rotT)
    sincos[DP : DP + DH] = sincos[DP + DH : DP + 2 * DH] = np.cos(rotT)
    in_maps = []
    for cc in range(NCORES):
        sl = slice(L * cc, L * (cc + 1))
        hTc = hT_full[:, sl]  # [1280, 1024]
        # [half, p, c(5), tok]
        hp = (
            hTc.reshape(2, 5, 128, L)
            .transpose(0, 2, 1, 3)
            .reshape(256, 5 * L)
        )
        m = dict(base)
        m["hT"] = np.ascontiguousarray(hp).astype(NPBF16)
        m["rot"] = np.ascontiguousarray(sincos[:, sl]).astype(NPBF16)
        in_maps.append(m)
    return in_maps


_NC = {}


def _get_nc(with_bias=False):
    if with_bias not in _NC:
        _NC[with_bias] = build_nc(with_bias)
    return _NC[with_bias]


def run(inputs, trace=False, trace_kwargs=None):
    from concourse.bass_utils import run_bass_kernel_spmd

    with_bias = bool(
        np.any(np.asarray(inputs["b_qkv"])) or np.any(np.asarray(inputs["b_proj"]))
    )
    nc = _get_nc(with_bias)
    in_maps = host_prep(inputs)
    kw = {}
    if trace:
        kw = dict(trace=True, **(trace_kwargs or {}))
        kw.setdefault("trace_cores", list(range(NCORES)))
    res = run_bass_kernel_spmd(nc, in_maps, list(range(NCORES)), **kw)
    outs = np.concatenate([res.results[i]["out"] for i in range(NCORES)], axis=0)
    return outs.astype(np.float32), res


def kernel(**inputs) -> np.ndarray:
    out, _ = run(inputs)
    return out

